# revision 39
# baseline (speedup 1.0000x reference)
"""Multi-head self-attention (B=4, S=2048, D=1024, H=16, causal) on 8 TRN2 cores.

Sharding: core = (batch b, head-group g) with b = core//2, g = core%2.
Each core computes Q/K/V projections for its batch restricted to its 8 heads
(column-parallel), causal flash attention for those heads, and a row-parallel
partial of the output projection. Host sums the two partials per batch and
adds the bias terms. Zero collectives.

v2 structure: one software-pipelined stream. The attention inner loop is
ACT-bound (exp), so the PE-heavy projection of chunk qb+1 and the
out-projection of qb-1 are emitted as paced filler closures between the
attention matmuls of q-block qb (engines execute in order — overlap is
created by emission order). Scores are pipelined one k-tile ahead of AV so
the PE never waits on exp. Evictions run on DVE with bias adds riding the
eviction (tensor_scalar_add); GPSIMD cannot touch PSUM. qT/kT/pt/v/
cT/wo are bf16 (full-rate PE at any free size, DVE 2x mask mult); x and
the QKV weights stay f32r for accuracy. 1/sqrt(dh) is folded into Wq.
"""

import numpy as np

B = 4
S = 2048
D = 1024
H = 16
DH = 64
HG = 8            # heads per core
E = HG * DH       # 512 features per head-group
P = 128
NCORES = 8

DC = D // P       # 8 d-chunks
EC = E // P       # 4 e-chunks per group
QB = S // 512     # 4 query blocks of 512
NEG = -1.0e9

_CACHE = {}


def _build_nc(phases=(1, 2, 3), rep=1):
    import contextlib
    import concourse.mybir as mybir
    from concourse import bacc
    from concourse.tile import TileContext

    f32 = mybir.dt.float32
    f32r = mybir.dt.float32r
    bf16 = mybir.dt.bfloat16

    nc = bacc.Bacc("TRN2", target_bir_lowering=False, name=f"mhsa_r{rep}")
    xT = nc.dram_tensor("xT", [D, S], bf16, kind="ExternalInput")
    wq = nc.dram_tensor("wq", [D, E], bf16, kind="ExternalInput")
    wk = nc.dram_tensor("wk", [D, E], bf16, kind="ExternalInput")
    wv = nc.dram_tensor("wv", [D, E], bf16, kind="ExternalInput")
    wo = nc.dram_tensor("wo", [E, D], bf16, kind="ExternalInput")
    bq = nc.dram_tensor("bq", [P, EC], f32, kind="ExternalInput")
    bk = nc.dram_tensor("bk", [P, EC], f32, kind="ExternalInput")
    cm = nc.dram_tensor("cm", [P, 4, 512], bf16, kind="ExternalInput")
    blk = nc.dram_tensor("blk", [2, P], f32r, kind="ExternalInput")
    outp = nc.dram_tensor("outp", [S, D], bf16, kind="ExternalOutput")

    with TileContext(nc) as tc:
        # rep>1 wraps the whole computation in a hardware loop: used only by
        # the timing harness to measure marginal per-execution HW time with
        # the fixed axon dispatch overhead cancelled. kernel() uses rep=1.
        rep_ctx = tc.For_i(0, rep) if rep > 1 else contextlib.nullcontext()
        with rep_ctx, \
             tc.tile_pool(name="persist", bufs=1) as persist, \
             tc.tile_pool(name="wpool", bufs=1) as wpool, \
             tc.tile_pool(name="xpool", bufs=3) as xpool, \
             tc.tile_pool(name="ptpool", bufs=7) as ptpool, \
             tc.tile_pool(name="normpool", bufs=4) as normpool, \
             tc.tile_pool(name="evict", bufs=4) as evict, \
             tc.tile_pool(name="dram", bufs=4, space="DRAM") as dram_pool, \
             tc.tile_pool(name="ps_big", bufs=2, space="PSUM") as ps_big, \
             tc.tile_pool(name="ps_s", bufs=2, space="PSUM") as ps_s_pool, \
             tc.tile_pool(name="ps_av", bufs=2, space="PSUM") as ps_av_pool:

            qT_all = persist.tile([P, EC, S], bf16)      # 2 MB
            kT_all = persist.tile([P, EC, S], bf16)      # 2 MB
            TT = S // P
            v_aug = persist.tile([P, TT, HG, DH + 1], bf16)
            cT_all = persist.tile([P, EC, S], bf16)      # 2 MB
            cm_sb = persist.tile([P, 4, 512], bf16)
            bq_sb = persist.tile([P, EC], f32)
            bk_sb = persist.tile([P, EC], f32)

            # ---- per-chunk x loads (chunk c = tokens [512c, 512c+512)) ----
            def load_x(c):
                xt = xpool.tile([P, DC, 512], bf16, tag="xt")
                for dc in range(DC):
                    nc.sync.dma_start(
                        xt[:, dc], xT[dc * P:(dc + 1) * P,
                                      c * 512:(c + 1) * 512])
                return xt

            # DMA emission order is first-use order: wq0 + the first half of
            # xt0 unblock the first projection matmul ~2us in; everything
            # else fills behind.
            wq_sbs = [wpool.tile([P, DC, P], bf16, tag=f"wq{ec}",
                                 name=f"wq_sb{ec}")
                      for ec in range(EC)]
            nc.sync.dma_start(
                wq_sbs[0], wq[:, 0:P].rearrange("(dc p) e -> p dc e", p=P))
            xt0 = xpool.tile([P, DC, 512], bf16, tag="xt")
            for dc in range(4):
                nc.sync.dma_start(xt0[:, dc], xT[dc * P:(dc + 1) * P, 0:512])
            for ec in range(1, EC):
                nc.sync.dma_start(
                    wq_sbs[ec], wq[:, ec * P:(ec + 1) * P]
                    .rearrange("(dc p) e -> p dc e", p=P))
            for dc in range(4, DC):
                nc.sync.dma_start(xt0[:, dc], xT[dc * P:(dc + 1) * P, 0:512])
            nc.sync.dma_start(bq_sb, bq.ap())
            nc.sync.dma_start(bk_sb, bk.ap())
            wk_sb = wpool.tile([P, DC, E], bf16, tag="wk")
            for ec in range(EC):
                nc.sync.dma_start(
                    wk_sb[:, :, ec * P:(ec + 1) * P],
                    wk[:, ec * P:(ec + 1) * P]
                    .rearrange("(dc p) e -> p dc e", p=P))
            wv_sb = wpool.tile([P, DC, E], bf16, tag="wv")
            nc.sync.dma_start(wv_sb, wv.rearrange("(dc p) e -> p dc e", p=P))
            xt1 = load_x(1)
            nc.sync.dma_start(cm_sb, cm.ap())
            wo_sb = wpool.tile([P, EC, D], bf16, tag="wo")
            nc.sync.dma_start(wo_sb, wo.rearrange("(dc p) e -> p dc e", p=P))

            # ones columns of v_aug (denominator rides the AV matmul)
            ones_bf = persist.tile([P, TT, HG], bf16)
            nc.vector.memset(ones_bf, 1.0)
            nc.vector.tensor_copy(v_aug[:, :, :, DH], ones_bf)

            # warm up the ACT exp table set during the prologue DMA window
            # so the first attention exp doesn't pay the ~2.7us table load
            warm = persist.tile([1, 16], f32)
            nc.vector.memset(warm, 0.0)
            warm2 = persist.tile([1, 16], bf16)
            nc.scalar.activation(warm2, warm,
                                 mybir.ActivationFunctionType.Exp)

            # block-ones lhsT for the reciprocal partition-broadcast matmul:
            # out[m, q] = recips[m // 64, q]
            blk2 = persist.tile([2, P], f32r)
            nc.sync.dma_start(blk2, blk.ap())

            # ---- projection of one chunk as paced filler closures ----
            def proj_steps(c, xt):
                ts_ = slice(c * 512, (c + 1) * 512)
                steps = []
                for wsel, b_sb, dst in (("q", bq_sb, qT_all),
                                        ("k", bk_sb, kT_all)):
                    for ec in range(EC):
                        cell = {}

                        def mk_a(wsel=wsel, ec=ec, cell=cell, xt=xt):
                            # K-split row-group pair into DISJOINT psum
                            # halves: alternating row groups hide every
                            # LDWEIGHTS under the other half's matmul
                            psA = ps_big.tile([P, 512], mybir.dt.float32,
                                              tag="pp", name="psA")
                            psB = ps_big.tile([P, 512], mybir.dt.float32,
                                              tag="pp", name="psB")
                            cell["ps"] = (psA, psB)
                            for dc in range(4):
                                lhsT = (wq_sbs[ec][:, dc] if wsel == "q"
                                        else wk_sb[:, dc,
                                                   ec * P:(ec + 1) * P])
                                nc.tensor.matmul(psA, lhsT[0:DH],
                                                 xt[0:DH, dc],
                                                 start=(dc == 0), stop=False,
                                                 tile_position=(0, 0))
                                nc.tensor.matmul(psB, lhsT[DH:P],
                                                 xt[DH:P, dc],
                                                 start=(dc == 0), stop=False,
                                                 tile_position=(64, 0))

                        def mk_b(wsel=wsel, ec=ec, b_sb=b_sb, dst=dst,
                                 cell=cell, xt=xt, ts_=ts_):
                            psA, psB = cell["ps"]
                            for dc in range(4, DC):
                                lhsT = (wq_sbs[ec][:, dc] if wsel == "q"
                                        else wk_sb[:, dc,
                                                   ec * P:(ec + 1) * P])
                                nc.tensor.matmul(psA, lhsT[0:DH],
                                                 xt[0:DH, dc],
                                                 start=False,
                                                 stop=(dc == DC - 1),
                                                 tile_position=(0, 0))
                                nc.tensor.matmul(psB, lhsT[DH:P],
                                                 xt[DH:P, dc],
                                                 start=False,
                                                 stop=(dc == DC - 1),
                                                 tile_position=(64, 0))
                            tmp = evict.tile([P, 512], f32, tag="pt")
                            nc.vector.tensor_scalar_add(
                                tmp, psA, b_sb[:, ec:ec + 1])
                            nc.vector.tensor_tensor(
                                dst[:, ec, ts_], psB, tmp,
                                mybir.AluOpType.add)

                        steps.append(mk_a)
                        steps.append(mk_b)
                for tb in range(4):
                    cell = {}

                    def mk_a(tb=tb, cell=cell, xt=xt):
                        ps = ps_big.tile([P, E], mybir.dt.float32, tag="pp")
                        cell["ps"] = ps
                        for dc in range(4):
                            nc.tensor.matmul(
                                ps, xt[:, dc, tb * P:(tb + 1) * P],
                                wv_sb[:, dc], start=(dc == 0), stop=False)

                    def mk_b(c=c, tb=tb, cell=cell, xt=xt):
                        ps = cell["ps"]
                        for dc in range(4, DC):
                            nc.tensor.matmul(
                                ps, xt[:, dc, tb * P:(tb + 1) * P],
                                wv_sb[:, dc], start=False,
                                stop=(dc == DC - 1))
                        nc.vector.tensor_copy(
                            v_aug[:, c * 4 + tb, :, 0:DH],
                            ps.rearrange("p (h d) -> p h d", h=HG))

                    steps.append(mk_a)
                    steps.append(mk_b)
                return steps

            # ---- out-projection of q-block qb as filler closures ----
            def outproj_steps(qb):
                steps = []
                for tb in range(qb * 4, qb * 4 + 4):
                    for eb in range(2):
                        def mk(tb=tb, eb=eb):
                            ps = ps_big.tile([P, 512], mybir.dt.float32,
                                             tag="pp")
                            for dc in range(EC):
                                nc.tensor.matmul(
                                    ps, cT_all[:, dc, tb * P:(tb + 1) * P],
                                    wo_sb[:, dc, eb * 512:(eb + 1) * 512],
                                    start=(dc == 0), stop=(dc == EC - 1))
                            o_sb = evict.tile([P, 512], bf16, tag="o")
                            nc.vector.tensor_copy(o_sb, ps)
                            nc.sync.dma_start(
                                outp.ap()[tb * P:(tb + 1) * P,
                                          eb * 512:(eb + 1) * 512], o_sb)
                        steps.append(mk)
                return steps

            # ---- attention for q-block qb with paced fillers ----
            # diagonal k-tile kt (j = kt - 4qb) starts at column 128j; bf16
            # matmuls are full-rate at any width so true causal widths.
            DSTART = (0, 128, 256, 384)

            def attention_qb(qb, fillers):
                nkt = qb * 4 + 4
                q0 = qb * 512
                qs = slice(q0, q0 + 512)
                n_iters = EC * nkt
                n_fill = len(fillers)
                fi = [0]
                it = [0]

                def pull_fillers():
                    it[0] += 1
                    want = (it[0] * n_fill) // n_iters
                    while fi[0] < want:
                        fillers[fi[0]]()
                        fi[0] += 1

                AV_LAG = 3      # AV trails scores/exp by 3 k-tiles so the
                                # exp->mask->AV chain never stalls the PE
                for hp in range(EC):
                    ps_av0 = ps_av_pool.tile([DH + 1, 512],
                                             mybir.dt.float32, tag="av")
                    ps_av1 = ps_av_pool.tile([DH + 1, 512],
                                             mybir.dt.float32, tag="av")
                    pend = []

                    def emit_av(last):
                        pkt, ppt, pd0 = pend.pop(0)
                        nc.tensor.matmul(
                            ps_av0[:, pd0:], v_aug[:, pkt, 2 * hp],
                            ppt[:, 0, pd0:],
                            start=(pkt == 0), stop=last)
                        nc.tensor.matmul(
                            ps_av1[:, pd0:], v_aug[:, pkt, 2 * hp + 1],
                            ppt[:, 1, pd0:],
                            start=(pkt == 0), stop=last)

                    for kt in range(nkt):
                        ks = slice(kt * P, (kt + 1) * P)
                        diag = kt >= qb * 4
                        d0 = DSTART[kt - qb * 4] if diag else 0
                        w = 512 - d0
                        qsd = slice(q0 + d0, q0 + 512)
                        ps_s = ps_s_pool.tile([P, 2, 512],
                                              mybir.dt.float32, tag="s")
                        nc.tensor.matmul(ps_s[:, 0, d0:],
                                         kT_all[0:DH, hp, ks],
                                         qT_all[0:DH, hp, qsd],
                                         start=True, stop=True,
                                         tile_position=(0, 0))
                        nc.tensor.matmul(ps_s[:, 1, d0:],
                                         kT_all[DH:P, hp, ks],
                                         qT_all[DH:P, hp, qsd],
                                         start=True, stop=True,
                                         tile_position=(64, 0))
                        pt = ptpool.tile([P, 2, 512], bf16, tag="pt")
                        nc.scalar.activation(
                            pt[:, :, d0:], ps_s[:, :, d0:],
                            mybir.ActivationFunctionType.Exp)
                        if diag:
                            j = kt - qb * 4
                            nc.vector.tensor_tensor(
                                pt[:, :, d0:], pt[:, :, d0:],
                                cm_sb[:, j, None, d0:]
                                .to_broadcast([P, 2, w]),
                                mybir.AluOpType.mult)
                        pend.append((kt, pt, d0))
                        pull_fillers()
                        if len(pend) > AV_LAG:
                            emit_av(False)
                    while pend:
                        emit_av(len(pend) == 1)
                    # evict + normalize: c = av[0:64] * (1/av[64]) broadcast
                    # over rows. Steady state uses a DRAM-hop broadcast (its
                    # latency hides behind fillers); the very last chain of
                    # qb3 — fully exposed before the epilogue out-proj — uses
                    # a tiny PE matmul (blk2^T @ recip2) instead, saving the
                    # DRAM round trip on the critical tail.
                    tail = (qb == QB - 1) and (hp == EC - 1)
                    if tail:
                        av_sbs = []
                        recip2 = normpool.tile([1, 2, 512], f32r, tag="r2")
                        for idx, ps_av in ((0, ps_av0), (1, ps_av1)):
                            av_sb = normpool.tile([DH + 1, 512], f32,
                                                  tag="avs")
                            nc.vector.tensor_copy(av_sb, ps_av)
                            with nc.allow_low_precision(
                                    reason="f32r==f32 bits; rate-tag only"):
                                nc.vector.reciprocal(recip2[:, idx],
                                                     av_sb[DH:DH + 1])
                            av_sbs.append(av_sb)
                        for idx in range(2):
                            rb = ps_big.tile([P, 512], mybir.dt.float32,
                                             tag="pp", name="rb")
                            nc.tensor.matmul(rb[0:DH], blk2[0:1, 0:DH],
                                             recip2[:, idx],
                                             start=True, stop=True)
                            nc.vector.tensor_tensor(
                                cT_all[idx * DH:(idx + 1) * DH, hp, qs],
                                av_sbs[idx][0:DH], rb[0:DH],
                                mybir.AluOpType.mult)
                    else:
                        for idx, ps_av in ((0, ps_av0), (1, ps_av1)):
                            av_sb = normpool.tile([DH + 1, 512], f32,
                                                  tag="avs")
                            nc.vector.tensor_copy(av_sb, ps_av)
                            recip = normpool.tile([1, 512], f32, tag="recip")
                            nc.vector.reciprocal(recip, av_sb[DH:DH + 1])
                            r_dram = dram_pool.tile([1, 512], f32, tag="rd")
                            nc.sync.dma_start(r_dram, recip)
                            r_rep = normpool.tile([DH, 512], f32, tag="rrep")
                            nc.sync.dma_start(
                                r_rep, r_dram.to_broadcast([DH, 512]))
                            nc.vector.tensor_tensor(
                                cT_all[idx * DH:(idx + 1) * DH, hp, qs],
                                av_sb[0:DH], r_rep, mybir.AluOpType.mult)
                # drain any unpulled fillers
                while fi[0] < n_fill:
                    fillers[fi[0]]()
                    fi[0] += 1

            # ---------------- the pipelined program ----------------
            # Filler assignment balances each window's PE load against its
            # ACT (exp) load: qb0-2 carry the next chunk's projections
            # (PE-heavy windows), qb3 — the only ACT-bound window — absorbs
            # all out-projections except the last q-block's.
            xts = {0: xt0, 1: xt1}
            for st in proj_steps(0, xts[0]):
                st()
            for qb in range(QB):
                nxt = qb + 2
                if nxt < QB:
                    xts[nxt] = load_x(nxt)
                fillers = []
                if qb + 1 < QB:
                    fillers += proj_steps(qb + 1, xts[qb + 1])
                    if qb == QB - 2:
                        fillers += outproj_steps(0)[:2]
                else:
                    fillers += outproj_steps(0)[2:]
                    for pqb in range(1, QB - 1):
                        fillers += outproj_steps(pqb)
                attention_qb(qb, fillers)
            for st in outproj_steps(QB - 1):
                st()

    nc.finalize()
    return nc


def make_in_maps(x, Wq, bq, Wk, bk, Wv, bv, Wo, bo, mask):
    """Build the 8 per-core input dicts (host-side shard + transform)."""
    x = np.asarray(x, dtype=np.float32)
    Wq = np.asarray(Wq, dtype=np.float32)
    Wk = np.asarray(Wk, dtype=np.float32)
    Wv = np.asarray(Wv, dtype=np.float32)
    Wo = np.asarray(Wo, dtype=np.float32)
    bqf = np.asarray(bq, dtype=np.float32)
    bkf = np.asarray(bk, dtype=np.float32)
    mask = np.asarray(mask)

    import ml_dtypes
    bf16 = ml_dtypes.bfloat16

    scale = 1.0 / np.sqrt(np.float32(DH))
    # torch convention y = x @ W.T: feed W.T with d_in on axis 0
    WqT = np.ascontiguousarray(Wq.T) * scale        # [D, D], scale folded
    WkT = np.ascontiguousarray(Wk.T)
    WvT = np.ascontiguousarray(Wv.T)
    WoT = np.ascontiguousarray(Wo.T).astype(bf16)   # [D, D]

    # causal diag mask tiles: tile j covers keys [q0+128j, q0+128j+128) for
    # query block [q0, q0+512); tril is translation-invariant.
    q0 = S - 512
    m2 = mask.reshape(S, S)
    cmt = np.empty((P, 4, 512), np.float32)
    for j in range(4):
        sub = m2[q0:q0 + 512, q0 + 128 * j:q0 + 128 * j + 128]  # [q, k]
        cmt[:, j, :] = np.where(sub.T != 0, 1.0, 0.0)
    cmt = cmt.astype(bf16)

    xTs = [np.ascontiguousarray(x[b].T).astype(bf16) for b in range(B)]
    wslices = {}
    for g in range(2):
        cols = slice(g * E, (g + 1) * E)
        wslices[g] = {
            "wq": np.ascontiguousarray(WqT[:, cols]).astype(bf16),
            "wk": np.ascontiguousarray(WkT[:, cols]).astype(bf16),
            "wv": np.ascontiguousarray(WvT[:, cols]).astype(bf16),
            "wo": np.ascontiguousarray(WoT[cols, :]),
            "bq": np.ascontiguousarray((bqf[cols] * scale).reshape(EC, P).T),
            "bk": np.ascontiguousarray(bkf[cols].reshape(EC, P).T),
        }
    blk2 = np.zeros((2, P), np.float32)
    blk2[0, 0:DH] = 1.0
    blk2[1, DH:P] = 1.0
    in_maps = []
    for core in range(NCORES):
        b, g = divmod(core, 2)
        in_maps.append({
            "xT": xTs[b],                                # [D, S]
            **wslices[g],
            "cm": cmt,
            "blk": blk2,
        })
    return in_maps


def assemble_output(results, bv, bo, Wo):
    """Sum per-batch partials and add the bias correction."""
    bv = np.asarray(bv, dtype=np.float32)
    bo = np.asarray(bo, dtype=np.float32)
    Wo = np.asarray(Wo, dtype=np.float32)
    # context bias bv contributes bv @ Wo.T (attn rows sum to 1)
    corr = (bo + bv @ Wo.T).astype(np.float32)      # [D]
    out = np.empty((B, S, D), np.float32)
    for b in range(B):
        out[b] = (results[2 * b]["outp"].astype(np.float32)
                  + results[2 * b + 1]["outp"].astype(np.float32) + corr)
    return out


def kernel(x, Wq, bq, Wk, bk, Wv, bv, Wo, bo, mask):
    from concourse.bass_utils import run_bass_kernel_spmd

    if "nc" not in _CACHE:
        _CACHE["nc"] = _build_nc()
    nc = _CACHE["nc"]
    in_maps = make_in_maps(x, Wq, bq, Wk, bk, Wv, bv, Wo, bo, mask)
    res = run_bass_kernel_spmd(nc, in_maps, core_ids=list(range(NCORES)))
    return assemble_output(res.results, bv, bo, Wo)


# revision 40
# speedup vs baseline: 1.1935x; 1.1935x over previous
"""Multi-head self-attention (B=4, S=2048, D=1024, H=16, causal) on 8 TRN2 cores.

Sharding: core = (batch b, head-group g) with b = core//2, g = core%2.
Each core computes Q/K/V projections for its batch restricted to its 8 heads
(column-parallel), causal flash attention for those heads, and a row-parallel
partial of the output projection. Host sums the two partials per batch and
adds the bias terms. Zero collectives.

v2 structure: one software-pipelined stream. The attention inner loop is
ACT-bound (exp), so the PE-heavy projection of chunk qb+1 and the
out-projection of qb-1 are emitted as paced filler closures between the
attention matmuls of q-block qb (engines execute in order — overlap is
created by emission order). Scores are pipelined one k-tile ahead of AV so
the PE never waits on exp. Evictions run on DVE with bias adds riding the
eviction (tensor_scalar_add); GPSIMD cannot touch PSUM. qT/kT/pt/v/
cT/wo are bf16 (full-rate PE at any free size, DVE 2x mask mult); x and
the QKV weights stay f32r for accuracy. 1/sqrt(dh) is folded into Wq.
"""

import numpy as np

B = 4
S = 2048
D = 1024
H = 16
DH = 64
HG = 8            # heads per core
E = HG * DH       # 512 features per head-group
P = 128
NCORES = 8

DC = D // P       # 8 d-chunks
EC = E // P       # 4 e-chunks per group
QB = S // 512     # 4 query blocks of 512
NEG = -1.0e9

_CACHE = {}


def _build_nc(phases=(1, 2, 3), rep=1):
    import contextlib
    import concourse.mybir as mybir
    from concourse import bacc
    from concourse.tile import TileContext

    f32 = mybir.dt.float32
    f32r = mybir.dt.float32r
    bf16 = mybir.dt.bfloat16

    nc = bacc.Bacc("TRN2", target_bir_lowering=False, name=f"mhsa_r{rep}")
    xT = nc.dram_tensor("xT", [D, S], bf16, kind="ExternalInput")
    wq = nc.dram_tensor("wq", [D, E], bf16, kind="ExternalInput")
    wk = nc.dram_tensor("wk", [D, E], bf16, kind="ExternalInput")
    wv = nc.dram_tensor("wv", [D, E], bf16, kind="ExternalInput")
    wo = nc.dram_tensor("wo", [E, D], bf16, kind="ExternalInput")
    bq = nc.dram_tensor("bq", [P, EC], f32, kind="ExternalInput")
    bk = nc.dram_tensor("bk", [P, EC], f32, kind="ExternalInput")
    cm = nc.dram_tensor("cm", [P, 4, 512], bf16, kind="ExternalInput")
    blk = nc.dram_tensor("blk", [2, P], f32r, kind="ExternalInput")
    outp = nc.dram_tensor("outp", [S, D], bf16, kind="ExternalOutput")

    with TileContext(nc) as tc:
        # rep>1 wraps the whole computation in a hardware loop: used only by
        # the timing harness to measure marginal per-execution HW time with
        # the fixed axon dispatch overhead cancelled. kernel() uses rep=1.
        rep_ctx = tc.For_i(0, rep) if rep > 1 else contextlib.nullcontext()
        with rep_ctx, \
             tc.tile_pool(name="persist", bufs=1) as persist, \
             tc.tile_pool(name="wpool", bufs=1) as wpool, \
             tc.tile_pool(name="xpool", bufs=3) as xpool, \
             tc.tile_pool(name="ptpool", bufs=7) as ptpool, \
             tc.tile_pool(name="normpool", bufs=4) as normpool, \
             tc.tile_pool(name="evict", bufs=4) as evict, \
             tc.tile_pool(name="dram", bufs=4, space="DRAM") as dram_pool, \
             tc.tile_pool(name="ps_big", bufs=2, space="PSUM") as ps_big, \
             tc.tile_pool(name="ps_s", bufs=2, space="PSUM") as ps_s_pool, \
             tc.tile_pool(name="ps_av", bufs=2, space="PSUM") as ps_av_pool:

            qT_all = persist.tile([P, EC, S], bf16)      # 2 MB
            kT_all = persist.tile([P, EC, S], bf16)      # 2 MB
            TT = S // P
            v_aug = persist.tile([P, TT, HG, DH + 1], bf16)
            cT_all = persist.tile([P, EC, S], bf16)      # 2 MB
            cm_sb = persist.tile([P, 4, 512], bf16)
            bq_sb = persist.tile([P, EC], f32)
            bk_sb = persist.tile([P, EC], f32)

            # ---- per-chunk x loads (chunk c = tokens [512c, 512c+512)) ----
            def load_x(c):
                xt = xpool.tile([P, DC, 512], bf16, tag="xt")
                for dc in range(DC):
                    nc.sync.dma_start(
                        xt[:, dc], xT[dc * P:(dc + 1) * P,
                                      c * 512:(c + 1) * 512])
                return xt

            # DMA emission order is first-use order: wq0 + the first half of
            # xt0 unblock the first projection matmul ~2us in; everything
            # else fills behind.
            wq_sbs = [wpool.tile([P, DC, P], bf16, tag=f"wq{ec}",
                                 name=f"wq_sb{ec}")
                      for ec in range(EC)]
            nc.sync.dma_start(
                wq_sbs[0], wq[:, 0:P].rearrange("(dc p) e -> p dc e", p=P))
            xt0 = xpool.tile([P, DC, 512], bf16, tag="xt")
            for dc in range(4):
                nc.sync.dma_start(xt0[:, dc], xT[dc * P:(dc + 1) * P, 0:512])
            for ec in range(1, EC):
                nc.sync.dma_start(
                    wq_sbs[ec], wq[:, ec * P:(ec + 1) * P]
                    .rearrange("(dc p) e -> p dc e", p=P))
            for dc in range(4, DC):
                nc.sync.dma_start(xt0[:, dc], xT[dc * P:(dc + 1) * P, 0:512])
            nc.sync.dma_start(bq_sb, bq.ap())
            nc.sync.dma_start(bk_sb, bk.ap())
            wk_sb = wpool.tile([P, DC, E], bf16, tag="wk")
            for ec in range(EC):
                nc.sync.dma_start(
                    wk_sb[:, :, ec * P:(ec + 1) * P],
                    wk[:, ec * P:(ec + 1) * P]
                    .rearrange("(dc p) e -> p dc e", p=P))
            wv_sb = wpool.tile([P, DC, E], bf16, tag="wv")
            nc.sync.dma_start(wv_sb, wv.rearrange("(dc p) e -> p dc e", p=P))
            xt1 = load_x(1)
            nc.sync.dma_start(cm_sb, cm.ap())
            wo_sb = wpool.tile([P, EC, D], bf16, tag="wo")
            nc.sync.dma_start(wo_sb, wo.rearrange("(dc p) e -> p dc e", p=P))

            # ones columns of v_aug (denominator rides the AV matmul)
            ones_bf = persist.tile([P, TT, HG], bf16)
            nc.vector.memset(ones_bf, 1.0)
            nc.vector.tensor_copy(v_aug[:, :, :, DH], ones_bf)

            # warm up the ACT exp table set during the prologue DMA window
            # so the first attention exp doesn't pay the ~2.7us table load
            warm = persist.tile([1, 16], f32)
            nc.vector.memset(warm, 0.0)
            warm2 = persist.tile([1, 16], bf16)
            nc.scalar.activation(warm2, warm,
                                 mybir.ActivationFunctionType.Exp)

            # block-ones lhsT for the reciprocal partition-broadcast matmul:
            # out[m, q] = recips[m // 64, q]
            blk2 = persist.tile([2, P], f32r)
            nc.sync.dma_start(blk2, blk.ap())

            # ---- projection of one chunk as paced filler closures ----
            def proj_steps(c, xt):
                ts_ = slice(c * 512, (c + 1) * 512)
                steps = []
                for wsel, b_sb, dst in (("q", bq_sb, qT_all),
                                        ("k", bk_sb, kT_all)):
                    for ec in range(EC):
                        cell = {}

                        def mk_a(wsel=wsel, ec=ec, cell=cell, xt=xt):
                            ps = ps_big.tile([P, 512], mybir.dt.float32,
                                             tag="pp")
                            cell["ps"] = ps
                            for dc in range(4):
                                lhsT = (wq_sbs[ec][:, dc] if wsel == "q"
                                        else wk_sb[:, dc,
                                                   ec * P:(ec + 1) * P])
                                nc.tensor.matmul(ps, lhsT, xt[:, dc],
                                                 start=(dc == 0), stop=False)

                        def mk_b(wsel=wsel, ec=ec, b_sb=b_sb, dst=dst,
                                 cell=cell, xt=xt, ts_=ts_):
                            ps = cell["ps"]
                            for dc in range(4, DC):
                                lhsT = (wq_sbs[ec][:, dc] if wsel == "q"
                                        else wk_sb[:, dc,
                                                   ec * P:(ec + 1) * P])
                                nc.tensor.matmul(ps, lhsT, xt[:, dc],
                                                 start=False,
                                                 stop=(dc == DC - 1))
                            nc.vector.tensor_scalar_add(
                                dst[:, ec, ts_], ps, b_sb[:, ec:ec + 1])

                        steps.append(mk_a)
                        steps.append(mk_b)
                for tb in range(4):
                    cell = {}

                    def mk_a(tb=tb, cell=cell, xt=xt):
                        ps = ps_big.tile([P, E], mybir.dt.float32, tag="pp")
                        cell["ps"] = ps
                        for dc in range(4):
                            nc.tensor.matmul(
                                ps, xt[:, dc, tb * P:(tb + 1) * P],
                                wv_sb[:, dc], start=(dc == 0), stop=False)

                    def mk_b(c=c, tb=tb, cell=cell, xt=xt):
                        ps = cell["ps"]
                        for dc in range(4, DC):
                            nc.tensor.matmul(
                                ps, xt[:, dc, tb * P:(tb + 1) * P],
                                wv_sb[:, dc], start=False,
                                stop=(dc == DC - 1))
                        nc.vector.tensor_copy(
                            v_aug[:, c * 4 + tb, :, 0:DH],
                            ps.rearrange("p (h d) -> p h d", h=HG))

                    steps.append(mk_a)
                    steps.append(mk_b)
                return steps

            # ---- out-projection of q-block qb as filler closures ----
            def outproj_steps(qb):
                steps = []
                for tb in range(qb * 4, qb * 4 + 4):
                    for eb in range(2):
                        def mk(tb=tb, eb=eb):
                            ps = ps_big.tile([P, 512], mybir.dt.float32,
                                             tag="pp")
                            for dc in range(EC):
                                nc.tensor.matmul(
                                    ps, cT_all[:, dc, tb * P:(tb + 1) * P],
                                    wo_sb[:, dc, eb * 512:(eb + 1) * 512],
                                    start=(dc == 0), stop=(dc == EC - 1))
                            o_sb = evict.tile([P, 512], bf16, tag="o")
                            nc.vector.tensor_copy(o_sb, ps)
                            nc.sync.dma_start(
                                outp.ap()[tb * P:(tb + 1) * P,
                                          eb * 512:(eb + 1) * 512], o_sb)
                        steps.append(mk)
                return steps

            # ---- attention for q-block qb with paced fillers ----
            # diagonal k-tile kt (j = kt - 4qb) starts at column 128j; bf16
            # matmuls are full-rate at any width so true causal widths.
            DSTART = (0, 128, 256, 384)

            def attention_qb(qb, fillers):
                nkt = qb * 4 + 4
                q0 = qb * 512
                qs = slice(q0, q0 + 512)
                n_iters = EC * nkt
                n_fill = len(fillers)
                fi = [0]
                it = [0]

                def pull_fillers():
                    it[0] += 1
                    want = (it[0] * n_fill) // n_iters
                    while fi[0] < want:
                        fillers[fi[0]]()
                        fi[0] += 1

                AV_LAG = 3      # AV trails scores/exp by 3 k-tiles so the
                                # exp->mask->AV chain never stalls the PE
                for hp in range(EC):
                    ps_av0 = ps_av_pool.tile([DH + 1, 512],
                                             mybir.dt.float32, tag="av")
                    ps_av1 = ps_av_pool.tile([DH + 1, 512],
                                             mybir.dt.float32, tag="av")
                    pend = []

                    def emit_av(last):
                        pkt, ppt, pd0 = pend.pop(0)
                        nc.tensor.matmul(
                            ps_av0[:, pd0:], v_aug[:, pkt, 2 * hp],
                            ppt[:, 0, pd0:],
                            start=(pkt == 0), stop=last)
                        nc.tensor.matmul(
                            ps_av1[:, pd0:], v_aug[:, pkt, 2 * hp + 1],
                            ppt[:, 1, pd0:],
                            start=(pkt == 0), stop=last)

                    for kt in range(nkt):
                        ks = slice(kt * P, (kt + 1) * P)
                        diag = kt >= qb * 4
                        d0 = DSTART[kt - qb * 4] if diag else 0
                        w = 512 - d0
                        qsd = slice(q0 + d0, q0 + 512)
                        ps_s = ps_s_pool.tile([P, 2, 512],
                                              mybir.dt.float32, tag="s")
                        nc.tensor.matmul(ps_s[:, 0, d0:],
                                         kT_all[0:DH, hp, ks],
                                         qT_all[0:DH, hp, qsd],
                                         start=True, stop=True,
                                         tile_position=(0, 0))
                        nc.tensor.matmul(ps_s[:, 1, d0:],
                                         kT_all[DH:P, hp, ks],
                                         qT_all[DH:P, hp, qsd],
                                         start=True, stop=True,
                                         tile_position=(64, 0))
                        pt = ptpool.tile([P, 2, 512], bf16, tag="pt")
                        nc.scalar.activation(
                            pt[:, :, d0:], ps_s[:, :, d0:],
                            mybir.ActivationFunctionType.Exp)
                        if diag:
                            j = kt - qb * 4
                            nc.vector.tensor_tensor(
                                pt[:, :, d0:], pt[:, :, d0:],
                                cm_sb[:, j, None, d0:]
                                .to_broadcast([P, 2, w]),
                                mybir.AluOpType.mult)
                        pend.append((kt, pt, d0))
                        pull_fillers()
                        if len(pend) > AV_LAG:
                            emit_av(False)
                    while pend:
                        emit_av(len(pend) == 1)
                    # evict + normalize: c = av[0:64] * (1/av[64]) broadcast
                    # over rows. Steady state uses a DRAM-hop broadcast (its
                    # latency hides behind fillers); the very last chain of
                    # qb3 — fully exposed before the epilogue out-proj — uses
                    # a tiny PE matmul (blk2^T @ recip2) instead, saving the
                    # DRAM round trip on the critical tail.
                    tail = (qb == QB - 1) and (hp == EC - 1)
                    if tail:
                        av_sbs = []
                        recip2 = normpool.tile([1, 2, 512], f32r, tag="r2")
                        for idx, ps_av in ((0, ps_av0), (1, ps_av1)):
                            av_sb = normpool.tile([DH + 1, 512], f32,
                                                  tag="avs")
                            nc.vector.tensor_copy(av_sb, ps_av)
                            with nc.allow_low_precision(
                                    reason="f32r==f32 bits; rate-tag only"):
                                nc.vector.reciprocal(recip2[:, idx],
                                                     av_sb[DH:DH + 1])
                            av_sbs.append(av_sb)
                        for idx in range(2):
                            rb = ps_big.tile([P, 512], mybir.dt.float32,
                                             tag="pp", name="rb")
                            nc.tensor.matmul(rb[0:DH], blk2[0:1, 0:DH],
                                             recip2[:, idx],
                                             start=True, stop=True)
                            nc.vector.tensor_tensor(
                                cT_all[idx * DH:(idx + 1) * DH, hp, qs],
                                av_sbs[idx][0:DH], rb[0:DH],
                                mybir.AluOpType.mult)
                    else:
                        for idx, ps_av in ((0, ps_av0), (1, ps_av1)):
                            av_sb = normpool.tile([DH + 1, 512], f32,
                                                  tag="avs")
                            nc.vector.tensor_copy(av_sb, ps_av)
                            recip = normpool.tile([1, 512], f32, tag="recip")
                            nc.vector.reciprocal(recip, av_sb[DH:DH + 1])
                            r_dram = dram_pool.tile([1, 512], f32, tag="rd")
                            nc.sync.dma_start(r_dram, recip)
                            r_rep = normpool.tile([DH, 512], f32, tag="rrep")
                            nc.sync.dma_start(
                                r_rep, r_dram.to_broadcast([DH, 512]))
                            nc.vector.tensor_tensor(
                                cT_all[idx * DH:(idx + 1) * DH, hp, qs],
                                av_sb[0:DH], r_rep, mybir.AluOpType.mult)
                # drain any unpulled fillers
                while fi[0] < n_fill:
                    fillers[fi[0]]()
                    fi[0] += 1

            # ---------------- the pipelined program ----------------
            # Filler assignment balances each window's PE load against its
            # ACT (exp) load: qb0-2 carry the next chunk's projections
            # (PE-heavy windows), qb3 — the only ACT-bound window — absorbs
            # all out-projections except the last q-block's.
            xts = {0: xt0, 1: xt1}
            for st in proj_steps(0, xts[0]):
                st()
            for qb in range(QB):
                nxt = qb + 2
                if nxt < QB:
                    xts[nxt] = load_x(nxt)
                fillers = []
                if qb + 1 < QB:
                    fillers += proj_steps(qb + 1, xts[qb + 1])
                    if qb == QB - 2:
                        fillers += outproj_steps(0)[:2]
                else:
                    fillers += outproj_steps(0)[2:]
                    for pqb in range(1, QB - 1):
                        fillers += outproj_steps(pqb)
                attention_qb(qb, fillers)
            for st in outproj_steps(QB - 1):
                st()

    nc.finalize()
    return nc


def make_in_maps(x, Wq, bq, Wk, bk, Wv, bv, Wo, bo, mask):
    """Build the 8 per-core input dicts (host-side shard + transform)."""
    x = np.asarray(x, dtype=np.float32)
    Wq = np.asarray(Wq, dtype=np.float32)
    Wk = np.asarray(Wk, dtype=np.float32)
    Wv = np.asarray(Wv, dtype=np.float32)
    Wo = np.asarray(Wo, dtype=np.float32)
    bqf = np.asarray(bq, dtype=np.float32)
    bkf = np.asarray(bk, dtype=np.float32)
    mask = np.asarray(mask)

    import ml_dtypes
    bf16 = ml_dtypes.bfloat16

    scale = 1.0 / np.sqrt(np.float32(DH))
    # torch convention y = x @ W.T: feed W.T with d_in on axis 0
    WqT = np.ascontiguousarray(Wq.T) * scale        # [D, D], scale folded
    WkT = np.ascontiguousarray(Wk.T)
    WvT = np.ascontiguousarray(Wv.T)
    WoT = np.ascontiguousarray(Wo.T).astype(bf16)   # [D, D]

    # causal diag mask tiles: tile j covers keys [q0+128j, q0+128j+128) for
    # query block [q0, q0+512); tril is translation-invariant.
    q0 = S - 512
    m2 = mask.reshape(S, S)
    cmt = np.empty((P, 4, 512), np.float32)
    for j in range(4):
        sub = m2[q0:q0 + 512, q0 + 128 * j:q0 + 128 * j + 128]  # [q, k]
        cmt[:, j, :] = np.where(sub.T != 0, 1.0, 0.0)
    cmt = cmt.astype(bf16)

    xTs = [np.ascontiguousarray(x[b].T).astype(bf16) for b in range(B)]
    wslices = {}
    for g in range(2):
        cols = slice(g * E, (g + 1) * E)
        wslices[g] = {
            "wq": np.ascontiguousarray(WqT[:, cols]).astype(bf16),
            "wk": np.ascontiguousarray(WkT[:, cols]).astype(bf16),
            "wv": np.ascontiguousarray(WvT[:, cols]).astype(bf16),
            "wo": np.ascontiguousarray(WoT[cols, :]),
            "bq": np.ascontiguousarray((bqf[cols] * scale).reshape(EC, P).T),
            "bk": np.ascontiguousarray(bkf[cols].reshape(EC, P).T),
        }
    blk2 = np.zeros((2, P), np.float32)
    blk2[0, 0:DH] = 1.0
    blk2[1, DH:P] = 1.0
    in_maps = []
    for core in range(NCORES):
        b, g = divmod(core, 2)
        in_maps.append({
            "xT": xTs[b],                                # [D, S]
            **wslices[g],
            "cm": cmt,
            "blk": blk2,
        })
    return in_maps


def assemble_output(results, bv, bo, Wo):
    """Sum per-batch partials and add the bias correction."""
    bv = np.asarray(bv, dtype=np.float32)
    bo = np.asarray(bo, dtype=np.float32)
    Wo = np.asarray(Wo, dtype=np.float32)
    # context bias bv contributes bv @ Wo.T (attn rows sum to 1)
    corr = (bo + bv @ Wo.T).astype(np.float32)      # [D]
    out = np.empty((B, S, D), np.float32)
    for b in range(B):
        out[b] = (results[2 * b]["outp"].astype(np.float32)
                  + results[2 * b + 1]["outp"].astype(np.float32) + corr)
    return out


def kernel(x, Wq, bq, Wk, bk, Wv, bv, Wo, bo, mask):
    from concourse.bass_utils import run_bass_kernel_spmd

    if "nc" not in _CACHE:
        _CACHE["nc"] = _build_nc()
    nc = _CACHE["nc"]
    in_maps = make_in_maps(x, Wq, bq, Wk, bk, Wv, bv, Wo, bo, mask)
    res = run_bass_kernel_spmd(nc, in_maps, core_ids=list(range(NCORES)))
    return assemble_output(res.results, bv, bo, Wo)
